# revision 10
# baseline (speedup 1.0000x reference)
"""Trainium2 Bass kernel for nn_CompressiveMemory_57750130262084.

The reference computes (B=8, S=4096, DK=DV=1024):
    sigma  = elu(query) + 1                                  [B,S,DK]
    memory = einsum('bkd,bsv->bkv', swap(sigma), value)      [B,DK,DV]
    z_norm = sum_s sigma                                     [B,DK]
    out    = einsum('bsd,bkv->bsv', sigma, memory)
           / einsum('bsd,bk->bs',  sigma, z_norm)[..., None]

Every einsum uses disjoint summed subscripts, so each factorises into
outer products of independent reductions:
    memory[b,k,v]    = z_norm[b,k] * VS[b,v]      with VS[b,v] = sum_s value[b,s,v]
    retrieved[b,s,v] = rs[b,s] * Z[b] * VS[b,v]   with rs = rowsum(sigma), Z = sum_k z_norm
    denom[b,s]       = rs[b,s] * Z[b]
    out[b,s,v]       = VS[b,v]                    (exactly; query cancels)

So the kernel is a column-sum of `value` over S; every output row b,s
is the same vector VS[b,:].  Sharding: data-parallel over batch, one
NeuronCore per batch element; each core reduces its 16.8 MB value
shard to the 4 KB row VS[b,:], and the host's unshard step broadcasts
that row over S (pure replication — no arithmetic).

Schedule per core (v7).  The 16 SDMA engines sustain ~360 GB/s
HBM->SBUF (2 NCs share a 716 GB/s stack), so the floor is the 16.8 MB
read ~47 us plus the NEFF prologue and a small reduction tail:
  - p-major input layout: partition p holds 32 CONTIGUOUS DRAM rows
    [32p, 32p+32); row placement is irrelevant (everything is summed).
  - input split across BOTH HWDGE engines (SP + Activation), rows
    0..15 / 16..31, transfers per engine of [4,4,4,2,1,.5,.5] rows:
    big transfers up front (fewer completions), small at the end so
    the final adds/matmuls start as soon as possible.
  - pair k = (sync row k, scalar row k+16) lands atomically; the DVE
    adds each pair into a tmp ring CASTING TO BF16, so the PE's
    PSUM-accumulating ones^T @ tmp (partition-reduce, ones is
    [128,1]) costs 1 HW pass per 512-bank instead of f32's 2.
    bf16 pair rounding contributes ~2e-4 relative error (tolerance
    is 2e-2; fp32 keeps the partition/psum accumulation exact).
  - the final row-pair arrives as two half-row pieces; per-bank
    pipeline with per-bank stop so bank 0 drains while bank 1's last
    0.25 MB is still in flight.
  - DVE drains PSUM [1,1024] to SBUF f32; one 4 KB DMA writes VS.
    No ACT compute op anywhere -> no activation-table load.
"""

import numpy as np

B, S, D = 8, 4096, 1024
P = 128                 # SBUF partitions
RPP = S // P            # 32 rows per partition (p-major layout)
# rows/partition per transfer, in units of QUARTER-rows (256 f32 = 1 KB
# per partition, per engine).  Big transfers up front (fewer completion
# events), tiny at the end: the last two are single quarters so the
# final add->matmul->drain chain after the last byte is as short as
# possible.
GROUPS_QUARTER = [8, 16, 16, 8, 4, 2, 1, 1]   # rows 2..15 of each half
GPSIMD_QUARTER = 8                            # rows 0-1 / 16-17 via SWDGE
TMP_SLOTS = 4
H = 512                 # PSUM bank width in f32 (matmul N limit)
Q = 256                 # quarter-row width in f32

_CACHE: dict = {}


def _build_program():
    import concourse.mybir as mybir
    import concourse.tile as tile
    from concourse import bacc

    f32 = mybir.dt.float32
    bf16 = mybir.dt.bfloat16
    assert GPSIMD_QUARTER + sum(GROUPS_QUARTER) == 64
    nc = bacc.Bacc("TRN2", target_bir_lowering=False, debug=False, num_devices=B, enable_asserts=False)
    v = nc.declare_dram_parameter("value", [S, D], f32, isOutput=False)
    o = nc.declare_dram_parameter("out", [1, D], f32, isOutput=True)

    v_pm = v[:].rearrange("(p r) m -> p (r m)", p=P)       # [128][32*1024]

    with tile.TileContext(nc) as tc:
        with (
            tc.tile_pool(name="in", bufs=1) as in_pool,
            tc.tile_pool(name="tmp", bufs=1) as tmp_pool,
            tc.tile_pool(name="ones", bufs=1) as ones_pool,
            tc.tile_pool(name="res", bufs=1) as res_pool,
            tc.tile_pool(name="psum", bufs=1, space="PSUM") as psum_pool,
        ):
            t = in_pool.tile([P, RPP * D], f32)
            tmp = tmp_pool.tile([P, TMP_SLOTS * D], bf16)
            ones = ones_pool.tile([P, 1], bf16)
            # Output quarter q lives in the FIRST 256 f32 of its own
            # PSUM bank (cols [512q, 512q+256) of this [1, 2048] tile),
            # so the final per-quarter drains never share a bank with a
            # still-accumulating matmul (PSUM deps are bank-granular).
            ps = psum_pool.tile([1, 4 * H], f32)

            # Input DMAs.  The first two rows of each half go through
            # SWDGE (gpsimd): its Q7-generated descriptors reach the
            # SDMA engines while the HWDGE RTL is still emitting the
            # first transfers' descriptors, filling the startup ramp.
            # The HWDGE engines (sync/scalar) then stream the rest
            # back-to-back.
            for half in (0, 1):
                q0 = half * 64                       # offset in quarter-rows
                sl = slice(q0 * Q, (q0 + GPSIMD_QUARTER) * Q)
                nc.gpsimd.dma_start(t[:, sl], v_pm[:, sl])
            for half, eng in ((0, nc.sync), (1, nc.scalar)):
                q0 = half * 64 + GPSIMD_QUARTER
                for g in GROUPS_QUARTER:
                    sl = slice(q0 * Q, (q0 + g) * Q)
                    eng.dma_start(t[:, sl], v_pm[:, sl])
                    q0 += g

            nc.vector.memset(ones[:], 1.0)

            # Pairs 0..14: DVE add (f32 -> bf16 tmp), PE accumulates
            # four 256-wide matmuls per pair, one per PSUM bank
            # (partition reduce via ones[128,1], 1 HW pass in bf16).
            for k in range(15):
                a = t[:, k * D : (k + 1) * D]
                b = t[:, (k + 16) * D : (k + 17) * D]
                tk = tmp[:, (k % TMP_SLOTS) * D : (k % TMP_SLOTS + 1) * D]
                nc.vector.tensor_add(tk, a, b)
                for q in range(4):
                    nc.tensor.matmul(
                        ps[:, q * H : q * H + Q],
                        ones[:],
                        tk[:, q * Q : (q + 1) * Q],
                        start=(k == 0),
                        stop=False,
                    )

            # Pair 15 arrives as one half-row piece (quarters 0-1) and
            # two quarter-row pieces; each quarter stops and drains its
            # OWN bank, so only the truly last 1 KB/partition piece's
            # add -> matmul -> copy chain is on the post-last-byte path.
            res = res_pool.tile([1, D], f32)
            for lo, w in ((0, H), (H, Q), (H + Q, Q)):
                a = t[:, 15 * D + lo : 15 * D + lo + w]
                b = t[:, 31 * D + lo : 31 * D + lo + w]
                th = tmp[:, 3 * D + lo : 3 * D + lo + w]
                nc.vector.tensor_add(th, a, b)
                for q in range(lo // Q, (lo + w) // Q):
                    nc.tensor.matmul(
                        ps[:, q * H : q * H + Q],
                        ones[:],
                        th[:, q * Q - lo : (q + 1) * Q - lo],
                        start=False,
                        stop=True,
                    )
                    nc.vector.tensor_copy(
                        res[:, q * Q : (q + 1) * Q], ps[:, q * H : q * H + Q]
                    )

            nc.sync.dma_start(o[:], res[:])

    nc.compile()
    return nc


def _get_program():
    if "nc" not in _CACHE:
        _CACHE["nc"] = _build_program()
    return _CACHE["nc"]


def kernel(query: np.ndarray, value: np.ndarray) -> np.ndarray:
    from concourse.bass_utils import run_bass_kernel_spmd

    del query  # output is exactly independent of query (see module docstring)
    value = np.ascontiguousarray(value, dtype=np.float32)
    assert value.shape == (B, S, D)

    nc = _get_program()
    in_maps = [{"value": value[b]} for b in range(B)]
    try:
        res = run_bass_kernel_spmd(nc, in_maps, list(range(B)))
    except Exception:
        # The tunneled runtime occasionally surfaces a transient
        # NRT_EXEC_UNIT_UNRECOVERABLE on the first dispatch; retry once.
        import time

        time.sleep(2.0)
        res = run_bass_kernel_spmd(nc, in_maps, list(range(B)))
    vs = np.stack([res.results[b]["out"].reshape(D) for b in range(B)], axis=0)
    # out[b, s, :] == VS[b, :] for every s — materialize the broadcast.
    return np.ascontiguousarray(
        np.broadcast_to(vs[:, None, :], (B, S, D)).astype(np.float32)
    )


# revision 13
# speedup vs baseline: 1.3139x; 1.3139x over previous
"""Trainium2 Bass kernel for nn_CompressiveMemory_57750130262084.

The reference computes (B=8, S=4096, DK=DV=1024):
    sigma  = elu(query) + 1                                  [B,S,DK]
    memory = einsum('bkd,bsv->bkv', swap(sigma), value)      [B,DK,DV]
    z_norm = sum_s sigma                                     [B,DK]
    out    = einsum('bsd,bkv->bsv', sigma, memory)
           / einsum('bsd,bk->bs',  sigma, z_norm)[..., None]

Every einsum uses disjoint summed subscripts, so each factorises into
outer products of independent reductions:
    memory[b,k,v]    = z_norm[b,k] * VS[b,v]      with VS[b,v] = sum_s value[b,s,v]
    retrieved[b,s,v] = rs[b,s] * Z[b] * VS[b,v]   with rs = rowsum(sigma), Z = sum_k z_norm
    denom[b,s]       = rs[b,s] * Z[b]
    out[b,s,v]       = VS[b,v]                    (exactly; query cancels)

So the kernel is a column-sum of `value` over S; every output row b,s
is the same vector VS[b,:].  Sharding: data-parallel over batch, one
NeuronCore per batch element; each core reduces its 16.8 MB value
shard to the 4 KB row VS[b,:], and the host's unshard step broadcasts
that row over S (pure replication — no arithmetic).

Schedule per core (v7).  The 16 SDMA engines sustain ~360 GB/s
HBM->SBUF (2 NCs share a 716 GB/s stack), so the floor is the 16.8 MB
read ~47 us plus the NEFF prologue and a small reduction tail:
  - p-major input layout: partition p holds 32 CONTIGUOUS DRAM rows
    [32p, 32p+32); row placement is irrelevant (everything is summed).
  - input split across BOTH HWDGE engines (SP + Activation), rows
    0..15 / 16..31, transfers per engine of [4,4,4,2,1,.5,.5] rows:
    big transfers up front (fewer completions), small at the end so
    the final adds/matmuls start as soon as possible.
  - pair k = (sync row k, scalar row k+16) lands atomically; the DVE
    adds each pair into a tmp ring CASTING TO BF16, so the PE's
    PSUM-accumulating ones^T @ tmp (partition-reduce, ones is
    [128,1]) costs 1 HW pass per 512-bank instead of f32's 2.
    bf16 pair rounding contributes ~2e-4 relative error (tolerance
    is 2e-2; fp32 keeps the partition/psum accumulation exact).
  - the final row-pair arrives as two half-row pieces; per-bank
    pipeline with per-bank stop so bank 0 drains while bank 1's last
    0.25 MB is still in flight.
  - DVE drains PSUM [1,1024] to SBUF f32; one 4 KB DMA writes VS.
    No ACT compute op anywhere -> no activation-table load.
"""

import numpy as np

B, S, D = 8, 4096, 1024
P = 128                 # SBUF partitions
RPP = S // P            # 32 rows per partition (p-major layout)
# rows/partition per transfer, in units of QUARTER-rows (256 f32 = 1 KB
# per partition, per engine).  Big transfers up front (fewer completion
# events), tiny at the end: the last two are single quarters so the
# final add->matmul->drain chain after the last byte is as short as
# possible.
GROUPS_QUARTER = [16, 16, 16, 8, 4, 2, 1, 1]
TMP_SLOTS = 4
H = 512                 # PSUM bank width in f32 (matmul N limit)
Q = 256                 # quarter-row width in f32

_CACHE: dict = {}


def _build_program():
    import concourse.mybir as mybir
    import concourse.tile as tile
    from concourse import bacc

    f32 = mybir.dt.float32
    bf16 = mybir.dt.bfloat16
    assert sum(GROUPS_QUARTER) == 64
    nc = bacc.Bacc("TRN2", target_bir_lowering=False, debug=False, num_devices=B, enable_asserts=False)
    v = nc.declare_dram_parameter("value", [S, D], f32, isOutput=False)
    o = nc.declare_dram_parameter("out", [1, D], f32, isOutput=True)

    v_pm = v[:].rearrange("(p r) m -> p (r m)", p=P)       # [128][32*1024]

    with tile.TileContext(nc) as tc:
        with (
            tc.tile_pool(name="in", bufs=1) as in_pool,
            tc.tile_pool(name="tmp", bufs=1) as tmp_pool,
            tc.tile_pool(name="ones", bufs=1) as ones_pool,
            tc.tile_pool(name="res", bufs=1) as res_pool,
            tc.tile_pool(name="psum", bufs=1, space="PSUM") as psum_pool,
        ):
            t = in_pool.tile([P, RPP * D], f32)
            tmp = tmp_pool.tile([P, TMP_SLOTS * D], bf16)
            ones = ones_pool.tile([P, 1], bf16)
            # Output quarter q lives in the FIRST 256 f32 of its own
            # PSUM bank (cols [512q, 512q+256) of this [1, 2048] tile),
            # so the final per-quarter drains never share a bank with a
            # still-accumulating matmul (PSUM deps are bank-granular).
            ps = psum_pool.tile([1, 4 * H], f32)

            # Input DMAs: each engine issues its transfers back-to-back.
            for half, eng in ((0, nc.sync), (1, nc.scalar)):
                q0 = half * 64                       # offset in quarter-rows
                for g in GROUPS_QUARTER:
                    sl = slice(q0 * Q, (q0 + g) * Q)
                    eng.dma_start(t[:, sl], v_pm[:, sl])
                    q0 += g

            nc.vector.memset(ones[:], 1.0)

            # Pairs 0..14: DVE add (f32 -> bf16 tmp), PE accumulates
            # four 256-wide matmuls per pair, one per PSUM bank
            # (partition reduce via ones[128,1], 1 HW pass in bf16).
            for k in range(15):
                a = t[:, k * D : (k + 1) * D]
                b = t[:, (k + 16) * D : (k + 17) * D]
                tk = tmp[:, (k % TMP_SLOTS) * D : (k % TMP_SLOTS + 1) * D]
                nc.vector.tensor_add(tk, a, b)
                for q in range(4):
                    nc.tensor.matmul(
                        ps[:, q * H : q * H + Q],
                        ones[:],
                        tk[:, q * Q : (q + 1) * Q],
                        start=(k == 0),
                        stop=False,
                    )

            # Pair 15 arrives as one half-row piece (quarters 0-1) and
            # two quarter-row pieces; each quarter stops and drains its
            # OWN bank, so only the truly last 1 KB/partition piece's
            # add -> matmul -> copy chain is on the post-last-byte path.
            res = res_pool.tile([1, D], f32)
            for lo, w in ((0, H), (H, Q), (H + Q, Q)):
                a = t[:, 15 * D + lo : 15 * D + lo + w]
                b = t[:, 31 * D + lo : 31 * D + lo + w]
                th = tmp[:, 3 * D + lo : 3 * D + lo + w]
                nc.vector.tensor_add(th, a, b)
                for q in range(lo // Q, (lo + w) // Q):
                    nc.tensor.matmul(
                        ps[:, q * H : q * H + Q],
                        ones[:],
                        th[:, q * Q - lo : (q + 1) * Q - lo],
                        start=False,
                        stop=True,
                    )
                    nc.vector.tensor_copy(
                        res[:, q * Q : (q + 1) * Q], ps[:, q * H : q * H + Q]
                    )

            nc.sync.dma_start(o[:], res[:])

    nc.compile()
    return nc


def _get_program():
    if "nc" not in _CACHE:
        _CACHE["nc"] = _build_program()
    return _CACHE["nc"]


def kernel(query: np.ndarray, value: np.ndarray) -> np.ndarray:
    from concourse.bass_utils import run_bass_kernel_spmd

    del query  # output is exactly independent of query (see module docstring)
    value = np.ascontiguousarray(value, dtype=np.float32)
    assert value.shape == (B, S, D)

    nc = _get_program()
    in_maps = [{"value": value[b]} for b in range(B)]
    try:
        res = run_bass_kernel_spmd(nc, in_maps, list(range(B)))
    except Exception:
        # The tunneled runtime occasionally surfaces a transient
        # NRT_EXEC_UNIT_UNRECOVERABLE on the first dispatch; retry once.
        import time

        time.sleep(2.0)
        res = run_bass_kernel_spmd(nc, in_maps, list(range(B)))
    vs = np.stack([res.results[b]["out"].reshape(D) for b in range(B)], axis=0)
    # out[b, s, :] == VS[b, :] for every s — materialize the broadcast.
    return np.ascontiguousarray(
        np.broadcast_to(vs[:, None, :], (B, S, D)).astype(np.float32)
    )


# revision 14
# speedup vs baseline: 1.3142x; 1.0002x over previous
"""Trainium2 Bass kernel for nn_CompressiveMemory_57750130262084.

The reference computes (B=8, S=4096, DK=DV=1024):
    sigma  = elu(query) + 1                                  [B,S,DK]
    memory = einsum('bkd,bsv->bkv', swap(sigma), value)      [B,DK,DV]
    z_norm = sum_s sigma                                     [B,DK]
    out    = einsum('bsd,bkv->bsv', sigma, memory)
           / einsum('bsd,bk->bs',  sigma, z_norm)[..., None]

Every einsum uses disjoint summed subscripts, so each factorises into
outer products of independent reductions:
    memory[b,k,v]    = z_norm[b,k] * VS[b,v]      with VS[b,v] = sum_s value[b,s,v]
    retrieved[b,s,v] = rs[b,s] * Z[b] * VS[b,v]   with rs = rowsum(sigma), Z = sum_k z_norm
    denom[b,s]       = rs[b,s] * Z[b]
    out[b,s,v]       = VS[b,v]                    (exactly; query cancels)

So the kernel is a column-sum of `value` over S; every output row b,s
is the same vector VS[b,:].  Sharding: data-parallel over batch, one
NeuronCore per batch element; each core reduces its 16.8 MB value
shard to the 4 KB row VS[b,:], and the host's unshard step broadcasts
that row over S (pure replication — no arithmetic).

Schedule per core (v9).  The 16 SDMA engines sustain ~360-430 GB/s
HBM->SBUF (2 NCs share a 716 GB/s stack; the exact rate depends on
how much the stack-neighbour core's stream overlaps), so the floor is
the 16.8 MB read (~40-47 us) plus the ~6.6 us NEFF prologue and a
~5 us tail (last-piece reduce chain + 4 KB out DMA + end barrier):
  - p-major input layout: partition p holds 32 CONTIGUOUS DRAM rows
    [32p, 32p+32); row placement is irrelevant (everything is summed).
  - input split across BOTH HWDGE engines (SP + Activation), rows
    0..15 / 16..31, transfers per engine of [4,4,4,2,1,.5,.25,.25]
    rows: big transfers up front (fewer completion events), tiny at
    the end so the post-last-byte chain is as short as possible.
  - pair k = (sync row k, scalar row k+16) lands atomically; the DVE
    adds each pair into a tmp ring CASTING TO BF16, so the PE's
    PSUM-accumulating ones^T @ tmp (partition-reduce, ones is
    [128,1]) costs 1 HW pass instead of f32's 2.  bf16 pair rounding
    contributes ~1.7e-3 relative error (tolerance is 2e-2; fp32
    keeps the partition/psum accumulation exact).
  - output quarter q accumulates in its OWN PSUM bank (4 x 256-wide
    matmuls per pair): PSUM dependencies are bank-granular, so each
    final piece's stop+drain never blocks a later piece's matmul.
  - the final row-pair arrives as one half-row piece plus two
    quarter-row pieces; each stops and drains its own bank, keeping
    only a ~1.8 us add->matmul->copy chain after the last byte.
  - DVE drains PSUM to SBUF f32; one 4 KB DMA writes VS.  No ACT
    compute op anywhere -> no activation-table load.
"""

import numpy as np

B, S, D = 8, 4096, 1024
P = 128                 # SBUF partitions
RPP = S // P            # 32 rows per partition (p-major layout)
# rows/partition per transfer, in units of QUARTER-rows (256 f32 = 1 KB
# per partition, per engine).  Big transfers up front (fewer completion
# events), tiny at the end: the last two are single quarters so the
# final add->matmul->drain chain after the last byte is as short as
# possible.
GROUPS_QUARTER = [16, 16, 16, 8, 4, 2, 1, 1]
TMP_SLOTS = 4
H = 512                 # PSUM bank width in f32 (matmul N limit)
Q = 256                 # quarter-row width in f32

_CACHE: dict = {}


def _build_program():
    import concourse.mybir as mybir
    import concourse.tile as tile
    from concourse import bacc

    f32 = mybir.dt.float32
    bf16 = mybir.dt.bfloat16
    assert sum(GROUPS_QUARTER) == 64
    nc = bacc.Bacc("TRN2", target_bir_lowering=False, debug=False, num_devices=B, enable_asserts=False)
    v = nc.declare_dram_parameter("value", [S, D], f32, isOutput=False)
    o = nc.declare_dram_parameter("out", [1, D], f32, isOutput=True)

    v_pm = v[:].rearrange("(p r) m -> p (r m)", p=P)       # [128][32*1024]

    with tile.TileContext(nc) as tc:
        with (
            tc.tile_pool(name="in", bufs=1) as in_pool,
            tc.tile_pool(name="tmp", bufs=1) as tmp_pool,
            tc.tile_pool(name="ones", bufs=1) as ones_pool,
            tc.tile_pool(name="res", bufs=1) as res_pool,
            tc.tile_pool(name="psum", bufs=1, space="PSUM") as psum_pool,
        ):
            t = in_pool.tile([P, RPP * D], f32)
            tmp = tmp_pool.tile([P, TMP_SLOTS * D], bf16)
            ones = ones_pool.tile([P, 1], bf16)
            # Output quarter q lives in the FIRST 256 f32 of its own
            # PSUM bank (cols [512q, 512q+256) of this [1, 2048] tile),
            # so the final per-quarter drains never share a bank with a
            # still-accumulating matmul (PSUM deps are bank-granular).
            ps = psum_pool.tile([1, 4 * H], f32)

            # Input DMAs: each engine issues its transfers back-to-back.
            for half, eng in ((0, nc.sync), (1, nc.scalar)):
                q0 = half * 64                       # offset in quarter-rows
                for g in GROUPS_QUARTER:
                    sl = slice(q0 * Q, (q0 + g) * Q)
                    eng.dma_start(t[:, sl], v_pm[:, sl])
                    q0 += g

            nc.vector.memset(ones[:], 1.0)

            # Pairs 0..14: DVE add (f32 -> bf16 tmp), PE accumulates
            # four 256-wide matmuls per pair, one per PSUM bank
            # (partition reduce via ones[128,1], 1 HW pass in bf16).
            for k in range(15):
                a = t[:, k * D : (k + 1) * D]
                b = t[:, (k + 16) * D : (k + 17) * D]
                tk = tmp[:, (k % TMP_SLOTS) * D : (k % TMP_SLOTS + 1) * D]
                nc.vector.tensor_add(tk, a, b)
                for q in range(4):
                    nc.tensor.matmul(
                        ps[:, q * H : q * H + Q],
                        ones[:],
                        tk[:, q * Q : (q + 1) * Q],
                        start=(k == 0),
                        stop=False,
                    )

            # Pair 15 arrives as one half-row piece (quarters 0-1) and
            # two quarter-row pieces; each quarter stops and drains its
            # OWN bank, so only the truly last 1 KB/partition piece's
            # add -> matmul -> copy chain is on the post-last-byte path.
            res = res_pool.tile([1, D], f32)
            for lo, w in ((0, H), (H, Q), (H + Q, Q)):
                a = t[:, 15 * D + lo : 15 * D + lo + w]
                b = t[:, 31 * D + lo : 31 * D + lo + w]
                th = tmp[:, 3 * D + lo : 3 * D + lo + w]
                nc.vector.tensor_add(th, a, b)
                for q in range(lo // Q, (lo + w) // Q):
                    nc.tensor.matmul(
                        ps[:, q * H : q * H + Q],
                        ones[:],
                        th[:, q * Q - lo : (q + 1) * Q - lo],
                        start=False,
                        stop=True,
                    )
                    nc.vector.tensor_copy(
                        res[:, q * Q : (q + 1) * Q], ps[:, q * H : q * H + Q]
                    )

            nc.sync.dma_start(o[:], res[:])

    nc.compile()
    return nc


def _get_program():
    if "nc" not in _CACHE:
        _CACHE["nc"] = _build_program()
    return _CACHE["nc"]


def kernel(query: np.ndarray, value: np.ndarray) -> np.ndarray:
    from concourse.bass_utils import run_bass_kernel_spmd

    del query  # output is exactly independent of query (see module docstring)
    value = np.ascontiguousarray(value, dtype=np.float32)
    assert value.shape == (B, S, D)

    nc = _get_program()
    in_maps = [{"value": value[b]} for b in range(B)]
    try:
        res = run_bass_kernel_spmd(nc, in_maps, list(range(B)))
    except Exception:
        # The tunneled runtime occasionally surfaces a transient
        # NRT_EXEC_UNIT_UNRECOVERABLE on the first dispatch; retry once.
        import time

        time.sleep(2.0)
        res = run_bass_kernel_spmd(nc, in_maps, list(range(B)))
    vs = np.stack([res.results[b]["out"].reshape(D) for b in range(B)], axis=0)
    # out[b, s, :] == VS[b, :] for every s — materialize the broadcast.
    return np.ascontiguousarray(
        np.broadcast_to(vs[:, None, :], (B, S, D)).astype(np.float32)
    )
